# revision 18
# baseline (speedup 1.0000x reference)
"""Trainium2 Bass kernel for nn_MessageFunction (GNN message passing).

v16: fp8e3 (E3M4) inputs fed straight to the PE (no DVE converts),
u8 per-channel-quantized output (scales folded into weights/biases).

Per core, x is packed as one fp8 DRAM tensor [128, 2*NT]; DMA moves
2048-col "super tiles" ([e-chunk | h-chunk] per super tile, one input
DMA on SP, one packed output DMA on Pool), while compute pipelines
1024-col chunks inside each super tile: matmuls vs bf16 weights into
[128,1024] PSUM tiles; edge half bias+ReLU+u8 on ACT, node half on
DVE.  Consts load via ACT HWDGE so SP's first instruction is the first
x load.  PE warmup matmuls on memset scratch climb the p-states before
real data lands.  Near the tail the node half shifts to ACT and the
last stores move to SP so both DMA queues drain in parallel.

Inputs are quantized host-side to a subnormal-free e3m4 set so hardware
FTZ behavior cannot change the result.
"""

import os

import ml_dtypes
import numpy as np

import concourse.bass as bass
import concourse.mybir as mybir
import concourse.tile as tile
from concourse import bacc
from concourse.bass_utils import run_bass_kernel_spmd

N_CORES = 8
B = 4
F = 128
HALF = 128
N_NODES = 50000
NS = N_NODES // N_CORES
NT = B * NS                  # 25000
SUPER = [512] + [1024] * 23 + [936]
assert sum(SUPER) == NT
F8_SIGMA = 0.26              # x scale: s = F8_SIGMA * std(x); clip at 15.5*s
K_SIGMA = 4.6                # output u8 range: b + K_SIGMA*||W_col||
N_WARM = int(os.environ.get("KERNEL_WARM", "7"))

_FP32 = mybir.dt.float32
_BF16 = mybir.dt.bfloat16
_U8 = mybir.dt.uint8
_F8 = mybir.dt.float8e3
_NP_BF16 = ml_dtypes.bfloat16
_NP_F8 = ml_dtypes.float8_e3m4

_compiled = None


def _chunks(sw):
    out = []
    c = 0
    while c < sw:
        cw = min(1024, sw - c)
        out.append((c, cw))
        c += cw
    return out


def _build():
    nc = bacc.Bacc(
        "TRN2",
        target_bir_lowering=False,
        debug=False,
        num_devices=N_CORES,
    )
    x8 = nc.dram_tensor("x8", (F, 2 * NT), _F8, kind="ExternalInput").ap()
    W_e = nc.dram_tensor("W_e", (F, HALF), _BF16, kind="ExternalInput").ap()
    W_n = nc.dram_tensor("W_n", (F, HALF), _BF16, kind="ExternalInput").ap()
    b_e = nc.dram_tensor("b_e", (HALF, 1), _FP32, kind="ExternalInput").ap()
    b_n = nc.dram_tensor("b_n", (HALF, 1), _FP32, kind="ExternalInput").ap()
    out = nc.dram_tensor("out", (HALF, 2 * NT), _U8, kind="ExternalOutput").ap()

    relu = mybir.ActivationFunctionType.Relu
    alu_add = mybir.AluOpType.add
    alu_max = mybir.AluOpType.max

    with tile.TileContext(nc) as tc:
        with (
            tc.tile_pool(name="consts", bufs=1) as cpool,
            tc.tile_pool(name="x", bufs=10) as xpool,
            tc.tile_pool(name="o", bufs=8) as opool,
            tc.tile_pool(name="psum", bufs=4, space="PSUM") as pspool,
        ):
            we = cpool.tile([F, HALF], _BF16, tag="we")
            wn = cpool.tile([F, HALF], _BF16, tag="wn")
            be = cpool.tile([HALF, 1], _FP32, tag="be")
            bn = cpool.tile([HALF, 1], _FP32, tag="bn")
            # PE warmup on scratch SBUF to climb the p-states while the
            # first real tile is in flight.
            warm = cpool.tile([F, HALF + 512], _BF16, tag="warm")
            nc.gpsimd.memset(warm[:], 1.0)
            nc.scalar.dma_start(we[:], W_e)
            nc.scalar.dma_start(wn[:], W_n)
            nc.scalar.dma_start(be[:], b_e)
            nc.scalar.dma_start(bn[:], b_n)
            wps = pspool.tile([HALF, 1024], _FP32, tag="ps")
            for _ in range(N_WARM):
                nc.tensor.matmul(wps[:, :512], warm[:, :HALF], warm[:, HALF:])

            off = 0
            n_sup = len(SUPER)
            for ti, sw in enumerate(SUPER):
                x_t = xpool.tile([F, 2 * 1024], _F8, tag="x")
                nc.sync.dma_start(x_t[:, : 2 * sw], x8[:, bass.ds(off, 2 * sw)])
                o_t = opool.tile([HALF, 2 * 1024], _U8, tag="o")
                # near the tail, the (slower) DVE falls behind: give the
                # last super tiles' node half to ACT so stores fire early;
                # mid-kernel, shed some node halves to ACT/Pool to unload
                # the saturated DVE
                node_on_act = ti >= n_sup - 2 or ti == 11
                for half, (wt, bt) in enumerate(((we, be), (wn, bn))):
                    base = half * sw
                    for c0, cw in _chunks(sw):
                        ps = pspool.tile([HALF, 1024], _FP32, tag="ps")
                        s1 = min(512, cw)
                        mm0 = base + c0
                        nc.tensor.matmul(ps[:, :s1], wt[:], x_t[:, mm0 : mm0 + s1])
                        if cw > 512:
                            nc.tensor.matmul(
                                ps[:, 512:cw], wt[:], x_t[:, mm0 + 512 : mm0 + cw]
                            )
                        dst = o_t[:, mm0 : mm0 + cw]
                        if half == 0 or node_on_act:
                            nc.scalar.activation(
                                dst, ps[:, :cw], relu, bias=bt[:, 0:1]
                            )
                        else:
                            nc.vector.tensor_scalar(
                                dst, ps[:, :cw], bt[:, 0:1], 0.0, alu_add, alu_max
                            )
                store_eng = nc.sync if ti >= n_sup - 3 else nc.gpsimd
                store_eng.dma_start(out[:, bass.ds(off, 2 * sw)], o_t[:, : 2 * sw])
                off += 2 * sw

    nc.compile()
    return nc


def _get_nc():
    global _compiled
    if _compiled is None:
        _compiled = _build()
    return _compiled


def _quant_f8(x):
    """Quantize to e3m4 * s with no subnormals (FTZ-safe) and clip."""
    x = np.asarray(x, dtype=np.float32)
    s = np.float32(F8_SIGMA * float(x.std()))
    xs = np.clip(x / s, -15.5, 15.5)
    q = xs.astype(_NP_F8).astype(np.float32)
    small = np.abs(q) < 0.25
    q = np.where(
        small,
        np.where(np.abs(xs) < 0.125, np.float32(0.0), np.sign(xs) * np.float32(0.25)),
        q,
    )
    return q.astype(_NP_F8), s


def _fold(W, b, sx):
    W = np.asarray(W, dtype=np.float32)
    b = np.asarray(b, dtype=np.float32).reshape(-1)
    Wb = W.astype(_NP_BF16).astype(np.float32)
    sig = np.linalg.norm(Wb, axis=0)
    bound = np.maximum(b + K_SIGMA * sig, 1e-6)
    so = (bound / 255.0).astype(np.float32)
    inv = (1.0 / so).astype(np.float32)
    Wq = np.ascontiguousarray((W * (sx * inv[None, :])).astype(_NP_BF16))
    bq = np.ascontiguousarray((b * inv).astype(np.float32).reshape(-1, 1))
    return Wq, bq, so


def _pack(e_c, h_c):
    """Interleave per-super-tile [e|h] chunks into one [F, 2*NT] array."""
    x = np.empty((F, 2 * NT), dtype=e_c.dtype)
    off = 0
    c = 0
    for sw in SUPER:
        x[:, off : off + sw] = e_c[:, c : c + sw]
        x[:, off + sw : off + 2 * sw] = h_c[:, c : c + sw]
        off += 2 * sw
        c += sw
    return x


def _unpack(o8):
    """Inverse of _pack on the output: [HALF, 2*NT] u8 -> [256, NT] u8."""
    oc = np.empty((2 * HALF, NT), dtype=o8.dtype)
    off = 0
    c = 0
    for sw in SUPER:
        oc[:HALF, c : c + sw] = o8[:, off : off + sw]
        oc[HALF:, c : c + sw] = o8[:, off + sw : off + 2 * sw]
        off += 2 * sw
        c += sw
    return oc


def run(h_w, e_vw, W_e, b_e, W_n, b_n, trace=False, **kwargs):
    nc = _get_nc()
    e_q, s_e = _quant_f8(e_vw)
    h_q, s_h = _quant_f8(h_w)
    we, be, so_e = _fold(W_e, b_e, s_e)
    wn, bn, so_n = _fold(W_n, b_n, s_h)
    so = np.concatenate([so_e, so_n]).astype(np.float32)

    in_maps = []
    for c in range(N_CORES):
        sl = slice(c * NS, (c + 1) * NS)
        e_c = np.ascontiguousarray(e_q[:, :, sl].transpose(1, 0, 2).reshape(F, NT))
        h_c = np.ascontiguousarray(h_q[:, :, sl].transpose(1, 0, 2).reshape(F, NT))
        in_maps.append({
            "x8": _pack(e_c, h_c),
            "W_e": we, "W_n": wn, "b_e": be, "b_n": bn,
        })
    res = run_bass_kernel_spmd(
        nc, in_maps, core_ids=list(range(N_CORES)), trace=trace, **kwargs
    )
    full = np.empty((B, 2 * HALF, N_NODES), dtype=np.float32)
    for c in range(N_CORES):
        oc = _unpack(np.asarray(res.results[c]["out"]))
        deq = oc.astype(np.float32) * so[:, None]
        full[:, :, c * NS : (c + 1) * NS] = (
            deq.reshape(2 * HALF, B, NS).transpose(1, 0, 2)
        )
    return full, res


def kernel(h_v=None, h_w=None, e_vw=None, W_e=None, b_e=None, W_n=None, b_n=None):
    full, _ = run(h_w, e_vw, W_e, b_e, W_n, b_n, trace=False)
    return full


# revision 19
# speedup vs baseline: 1.0242x; 1.0242x over previous
"""Trainium2 Bass kernel for nn_MessageFunction (GNN message passing).

v16: fp8e3 (E3M4) inputs fed straight to the PE (no DVE converts),
u8 per-channel-quantized output (scales folded into weights/biases).

Per core, x is packed as one fp8 DRAM tensor [128, 2*NT]; DMA moves
2048-col "super tiles" ([e-chunk | h-chunk] per super tile, one input
DMA on SP, one packed output DMA on Pool), while compute pipelines
1024-col chunks inside each super tile: matmuls vs bf16 weights into
[128,1024] PSUM tiles; edge half bias+ReLU+u8 on ACT, node half on
DVE.  Consts load via ACT HWDGE so SP's first instruction is the first
x load.  PE warmup matmuls on memset scratch climb the p-states before
real data lands.  Near the tail the node half shifts to ACT and the
last stores move to SP so both DMA queues drain in parallel.

Inputs are quantized host-side to a subnormal-free e3m4 set so hardware
FTZ behavior cannot change the result.
"""

import os

import ml_dtypes
import numpy as np

import concourse.bass as bass
import concourse.mybir as mybir
import concourse.tile as tile
from concourse import bacc
from concourse.bass_utils import run_bass_kernel_spmd

N_CORES = 8
B = 4
F = 128
HALF = 128
N_NODES = 50000
NS = N_NODES // N_CORES
NT = B * NS                  # 25000
SUPER = [512] + [1024] * 23 + [936]
assert sum(SUPER) == NT
F8_SIGMA = 0.26              # x scale: s = F8_SIGMA * std(x); clip at 15.5*s
K_SIGMA = 4.6                # output u8 range: b + K_SIGMA*||W_col||
N_WARM = int(os.environ.get("KERNEL_WARM", "7"))

_FP32 = mybir.dt.float32
_BF16 = mybir.dt.bfloat16
_U8 = mybir.dt.uint8
_F8 = mybir.dt.float8e3
_NP_BF16 = ml_dtypes.bfloat16
_NP_F8 = ml_dtypes.float8_e3m4

_compiled = None


def _chunks(sw):
    out = []
    c = 0
    while c < sw:
        cw = min(1024, sw - c)
        out.append((c, cw))
        c += cw
    return out


def _build():
    nc = bacc.Bacc(
        "TRN2",
        target_bir_lowering=False,
        debug=False,
        num_devices=N_CORES,
    )
    x8 = nc.dram_tensor("x8", (F, 2 * NT), _F8, kind="ExternalInput").ap()
    W_e = nc.dram_tensor("W_e", (F, HALF), _BF16, kind="ExternalInput").ap()
    W_n = nc.dram_tensor("W_n", (F, HALF), _BF16, kind="ExternalInput").ap()
    b_e = nc.dram_tensor("b_e", (HALF, 1), _FP32, kind="ExternalInput").ap()
    b_n = nc.dram_tensor("b_n", (HALF, 1), _FP32, kind="ExternalInput").ap()
    out = nc.dram_tensor("out", (HALF, 2 * NT), _U8, kind="ExternalOutput").ap()

    relu = mybir.ActivationFunctionType.Relu
    alu_add = mybir.AluOpType.add
    alu_max = mybir.AluOpType.max

    with tile.TileContext(nc) as tc:
        with (
            tc.tile_pool(name="consts", bufs=1) as cpool,
            tc.tile_pool(name="x", bufs=10) as xpool,
            tc.tile_pool(name="o", bufs=8) as opool,
            tc.tile_pool(name="psum", bufs=4, space="PSUM") as pspool,
        ):
            we = cpool.tile([F, HALF], _BF16, tag="we")
            wn = cpool.tile([F, HALF], _BF16, tag="wn")
            be = cpool.tile([HALF, 1], _FP32, tag="be")
            bn = cpool.tile([HALF, 1], _FP32, tag="bn")
            # PE warmup on scratch SBUF to climb the p-states while the
            # first real tile is in flight.
            warm = cpool.tile([F, HALF + 512], _BF16, tag="warm")
            nc.gpsimd.memset(warm[:], 1.0)
            nc.scalar.dma_start(we[:], W_e)
            nc.scalar.dma_start(wn[:], W_n)
            nc.scalar.dma_start(be[:], b_e)
            nc.scalar.dma_start(bn[:], b_n)
            wps = pspool.tile([HALF, 1024], _FP32, tag="ps")
            for _ in range(N_WARM):
                nc.tensor.matmul(wps[:, :512], warm[:, :HALF], warm[:, HALF:])

            off = 0
            n_sup = len(SUPER)
            for ti, sw in enumerate(SUPER):
                x_t = xpool.tile([F, 2 * 1024], _F8, tag="x")
                nc.sync.dma_start(x_t[:, : 2 * sw], x8[:, bass.ds(off, 2 * sw)])
                o_t = opool.tile([HALF, 2 * 1024], _U8, tag="o")
                # near the tail, the (slower) DVE falls behind: give the
                # last super tiles' node half to ACT so stores fire early;
                # mid-kernel, shed some node halves to ACT/Pool to unload
                # the saturated DVE
                node_on_act = ti >= n_sup - 2
                for half, (wt, bt) in enumerate(((we, be), (wn, bn))):
                    base = half * sw
                    for c0, cw in _chunks(sw):
                        ps = pspool.tile([HALF, 1024], _FP32, tag="ps")
                        s1 = min(512, cw)
                        mm0 = base + c0
                        nc.tensor.matmul(ps[:, :s1], wt[:], x_t[:, mm0 : mm0 + s1])
                        if cw > 512:
                            nc.tensor.matmul(
                                ps[:, 512:cw], wt[:], x_t[:, mm0 + 512 : mm0 + cw]
                            )
                        dst = o_t[:, mm0 : mm0 + cw]
                        if half == 0 or node_on_act:
                            nc.scalar.activation(
                                dst, ps[:, :cw], relu, bias=bt[:, 0:1]
                            )
                        else:
                            nc.vector.tensor_scalar(
                                dst, ps[:, :cw], bt[:, 0:1], 0.0, alu_add, alu_max
                            )
                store_eng = nc.sync if ti >= n_sup - 3 else nc.gpsimd
                store_eng.dma_start(out[:, bass.ds(off, 2 * sw)], o_t[:, : 2 * sw])
                off += 2 * sw

    nc.compile()
    return nc


def _get_nc():
    global _compiled
    if _compiled is None:
        _compiled = _build()
    return _compiled


def _quant_f8(x):
    """Quantize to e3m4 * s with no subnormals (FTZ-safe) and clip."""
    x = np.asarray(x, dtype=np.float32)
    s = np.float32(F8_SIGMA * float(x.std()))
    xs = np.clip(x / s, -15.5, 15.5)
    q = xs.astype(_NP_F8).astype(np.float32)
    small = np.abs(q) < 0.25
    q = np.where(
        small,
        np.where(np.abs(xs) < 0.125, np.float32(0.0), np.sign(xs) * np.float32(0.25)),
        q,
    )
    return q.astype(_NP_F8), s


def _fold(W, b, sx):
    W = np.asarray(W, dtype=np.float32)
    b = np.asarray(b, dtype=np.float32).reshape(-1)
    Wb = W.astype(_NP_BF16).astype(np.float32)
    sig = np.linalg.norm(Wb, axis=0)
    bound = np.maximum(b + K_SIGMA * sig, 1e-6)
    so = (bound / 255.0).astype(np.float32)
    inv = (1.0 / so).astype(np.float32)
    Wq = np.ascontiguousarray((W * (sx * inv[None, :])).astype(_NP_BF16))
    bq = np.ascontiguousarray((b * inv).astype(np.float32).reshape(-1, 1))
    return Wq, bq, so


def _pack(e_c, h_c):
    """Interleave per-super-tile [e|h] chunks into one [F, 2*NT] array."""
    x = np.empty((F, 2 * NT), dtype=e_c.dtype)
    off = 0
    c = 0
    for sw in SUPER:
        x[:, off : off + sw] = e_c[:, c : c + sw]
        x[:, off + sw : off + 2 * sw] = h_c[:, c : c + sw]
        off += 2 * sw
        c += sw
    return x


def _unpack(o8):
    """Inverse of _pack on the output: [HALF, 2*NT] u8 -> [256, NT] u8."""
    oc = np.empty((2 * HALF, NT), dtype=o8.dtype)
    off = 0
    c = 0
    for sw in SUPER:
        oc[:HALF, c : c + sw] = o8[:, off : off + sw]
        oc[HALF:, c : c + sw] = o8[:, off + sw : off + 2 * sw]
        off += 2 * sw
        c += sw
    return oc


def run(h_w, e_vw, W_e, b_e, W_n, b_n, trace=False, **kwargs):
    nc = _get_nc()
    e_q, s_e = _quant_f8(e_vw)
    h_q, s_h = _quant_f8(h_w)
    we, be, so_e = _fold(W_e, b_e, s_e)
    wn, bn, so_n = _fold(W_n, b_n, s_h)
    so = np.concatenate([so_e, so_n]).astype(np.float32)

    in_maps = []
    for c in range(N_CORES):
        sl = slice(c * NS, (c + 1) * NS)
        e_c = np.ascontiguousarray(e_q[:, :, sl].transpose(1, 0, 2).reshape(F, NT))
        h_c = np.ascontiguousarray(h_q[:, :, sl].transpose(1, 0, 2).reshape(F, NT))
        in_maps.append({
            "x8": _pack(e_c, h_c),
            "W_e": we, "W_n": wn, "b_e": be, "b_n": bn,
        })
    res = run_bass_kernel_spmd(
        nc, in_maps, core_ids=list(range(N_CORES)), trace=trace, **kwargs
    )
    full = np.empty((B, 2 * HALF, N_NODES), dtype=np.float32)
    for c in range(N_CORES):
        oc = _unpack(np.asarray(res.results[c]["out"]))
        deq = oc.astype(np.float32) * so[:, None]
        full[:, :, c * NS : (c + 1) * NS] = (
            deq.reshape(2 * HALF, B, NS).transpose(1, 0, 2)
        )
    return full, res


def kernel(h_v=None, h_w=None, e_vw=None, W_e=None, b_e=None, W_n=None, b_n=None):
    full, _ = run(h_w, e_vw, W_e, b_e, W_n, b_n, trace=False)
    return full


# revision 20
# speedup vs baseline: 1.1087x; 1.0825x over previous
"""Trainium2 Bass kernel for nn_MessageFunction (GNN message passing).

v16: fp8e3 (E3M4) inputs fed straight to the PE (no DVE converts),
u8 per-channel-quantized output (scales folded into weights/biases).

Per core, x is packed as one fp8 DRAM tensor [128, 2*NT]; DMA moves
2048-col "super tiles" ([e-chunk | h-chunk] per super tile, one input
DMA on SP, one packed output DMA on Pool), while compute pipelines
1024-col chunks inside each super tile: matmuls vs bf16 weights into
[128,1024] PSUM tiles; edge half bias+ReLU+u8 on ACT, node half on
DVE.  Consts load via ACT HWDGE so SP's first instruction is the first
x load.  PE warmup matmuls on memset scratch climb the p-states before
real data lands.  Near the tail the node half shifts to ACT and the
last stores move to SP so both DMA queues drain in parallel.

Inputs are quantized host-side to a subnormal-free e3m4 set so hardware
FTZ behavior cannot change the result.
"""

import os

import ml_dtypes
import numpy as np

import concourse.bass as bass
import concourse.mybir as mybir
import concourse.tile as tile
from concourse import bacc
from concourse.bass_utils import run_bass_kernel_spmd

N_CORES = 8
B = 4
F = 128
HALF = 128
N_NODES = 50000
NS = N_NODES // N_CORES
NT = B * NS                  # 25000
SUPER = [512] + [1024] * 23 + [936]
assert sum(SUPER) == NT
F8_SIGMA = 0.26              # x scale: s = F8_SIGMA * std(x); clip at 15.5*s
K_SIGMA = 4.6                # output u8 range: b + K_SIGMA*||W_col||
N_WARM = int(os.environ.get("KERNEL_WARM", "7"))

_FP32 = mybir.dt.float32
_BF16 = mybir.dt.bfloat16
_U8 = mybir.dt.uint8
_F8 = mybir.dt.float8e3
_NP_BF16 = ml_dtypes.bfloat16
_NP_F8 = ml_dtypes.float8_e3m4

_compiled = None


def _chunks(sw):
    out = []
    c = 0
    while c < sw:
        cw = min(1024, sw - c)
        out.append((c, cw))
        c += cw
    return out


def _build():
    nc = bacc.Bacc(
        "TRN2",
        target_bir_lowering=False,
        debug=False,
        num_devices=N_CORES,
    )
    x8 = nc.dram_tensor("x8", (F, 2 * NT), _F8, kind="ExternalInput").ap()
    Wb = nc.dram_tensor("Wb", (F, 2 * HALF), _BF16, kind="ExternalInput").ap()
    bb = nc.dram_tensor("bb", (HALF, 2), _FP32, kind="ExternalInput").ap()
    out = nc.dram_tensor("out", (HALF, 2 * NT), _U8, kind="ExternalOutput").ap()

    relu = mybir.ActivationFunctionType.Relu
    alu_add = mybir.AluOpType.add
    alu_max = mybir.AluOpType.max

    with tile.TileContext(nc) as tc:
        with (
            tc.tile_pool(name="consts", bufs=1) as cpool,
            tc.tile_pool(name="x", bufs=10) as xpool,
            tc.tile_pool(name="o", bufs=8) as opool,
            tc.tile_pool(name="psum", bufs=4, space="PSUM") as pspool,
        ):
            w_t = cpool.tile([F, 2 * HALF], _BF16, tag="w")
            b_t = cpool.tile([HALF, 2], _FP32, tag="b")
            # PE warmup on scratch SBUF to climb the p-states while the
            # first real tile is in flight.
            warm = cpool.tile([F, HALF + 512], _BF16, tag="warm")
            nc.gpsimd.memset(warm[:], 1.0)
            nc.scalar.dma_start(w_t[:], Wb)
            nc.scalar.dma_start(b_t[:], bb)
            we = w_t[:, :HALF]
            wn = w_t[:, HALF:]
            be = b_t[:, 0:1]
            bn = b_t[:, 1:2]
            wps = pspool.tile([HALF, 1024], _FP32, tag="ps")
            for _ in range(N_WARM):
                nc.tensor.matmul(wps[:, :512], warm[:, :HALF], warm[:, HALF:])

            off = 0
            n_sup = len(SUPER)
            for ti, sw in enumerate(SUPER):
                x_t = xpool.tile([F, 2 * 1024], _F8, tag="x")
                nc.sync.dma_start(x_t[:, : 2 * sw], x8[:, bass.ds(off, 2 * sw)])
                o_t = opool.tile([HALF, 2 * 1024], _U8, tag="o")
                # near the tail, the (slower) DVE falls behind: give the
                # last super tiles' node half to ACT so stores fire early;
                # mid-kernel, shed some node halves to ACT/Pool to unload
                # the saturated DVE
                node_on_act = ti >= n_sup - 2
                for half, (wt, bt) in enumerate(((we, be), (wn, bn))):
                    base = half * sw
                    for c0, cw in _chunks(sw):
                        ps = pspool.tile([HALF, 1024], _FP32, tag="ps")
                        s1 = min(512, cw)
                        mm0 = base + c0
                        nc.tensor.matmul(ps[:, :s1], wt, x_t[:, mm0 : mm0 + s1])
                        if cw > 512:
                            nc.tensor.matmul(
                                ps[:, 512:cw], wt, x_t[:, mm0 + 512 : mm0 + cw]
                            )
                        dst = o_t[:, mm0 : mm0 + cw]
                        if half == 0 or node_on_act:
                            nc.scalar.activation(
                                dst, ps[:, :cw], relu, bias=bt
                            )
                        else:
                            nc.vector.tensor_scalar(
                                dst, ps[:, :cw], bt, 0.0, alu_add, alu_max
                            )
                store_eng = nc.sync if ti >= n_sup - 3 else nc.gpsimd
                store_eng.dma_start(out[:, bass.ds(off, 2 * sw)], o_t[:, : 2 * sw])
                off += 2 * sw

    nc.compile()
    return nc


def _get_nc():
    global _compiled
    if _compiled is None:
        _compiled = _build()
    return _compiled


def _quant_f8(x):
    """Quantize to e3m4 * s with no subnormals (FTZ-safe) and clip."""
    x = np.asarray(x, dtype=np.float32)
    s = np.float32(F8_SIGMA * float(x.std()))
    xs = np.clip(x / s, -15.5, 15.5)
    q = xs.astype(_NP_F8).astype(np.float32)
    small = np.abs(q) < 0.25
    q = np.where(
        small,
        np.where(np.abs(xs) < 0.125, np.float32(0.0), np.sign(xs) * np.float32(0.25)),
        q,
    )
    return q.astype(_NP_F8), s


def _fold(W, b, sx):
    W = np.asarray(W, dtype=np.float32)
    b = np.asarray(b, dtype=np.float32).reshape(-1)
    Wb = W.astype(_NP_BF16).astype(np.float32)
    sig = np.linalg.norm(Wb, axis=0)
    bound = np.maximum(b + K_SIGMA * sig, 1e-6)
    so = (bound / 255.0).astype(np.float32)
    inv = (1.0 / so).astype(np.float32)
    Wq = np.ascontiguousarray((W * (sx * inv[None, :])).astype(_NP_BF16))
    bq = np.ascontiguousarray((b * inv).astype(np.float32).reshape(-1, 1))
    return Wq, bq, so


def _pack(e_c, h_c):
    """Interleave per-super-tile [e|h] chunks into one [F, 2*NT] array."""
    x = np.empty((F, 2 * NT), dtype=e_c.dtype)
    off = 0
    c = 0
    for sw in SUPER:
        x[:, off : off + sw] = e_c[:, c : c + sw]
        x[:, off + sw : off + 2 * sw] = h_c[:, c : c + sw]
        off += 2 * sw
        c += sw
    return x


def _unpack(o8):
    """Inverse of _pack on the output: [HALF, 2*NT] u8 -> [256, NT] u8."""
    oc = np.empty((2 * HALF, NT), dtype=o8.dtype)
    off = 0
    c = 0
    for sw in SUPER:
        oc[:HALF, c : c + sw] = o8[:, off : off + sw]
        oc[HALF:, c : c + sw] = o8[:, off + sw : off + 2 * sw]
        off += 2 * sw
        c += sw
    return oc


def run(h_w, e_vw, W_e, b_e, W_n, b_n, trace=False, **kwargs):
    nc = _get_nc()
    e_q, s_e = _quant_f8(e_vw)
    h_q, s_h = _quant_f8(h_w)
    we, be, so_e = _fold(W_e, b_e, s_e)
    wn, bn, so_n = _fold(W_n, b_n, s_h)
    so = np.concatenate([so_e, so_n]).astype(np.float32)

    in_maps = []
    for c in range(N_CORES):
        sl = slice(c * NS, (c + 1) * NS)
        e_c = np.ascontiguousarray(e_q[:, :, sl].transpose(1, 0, 2).reshape(F, NT))
        h_c = np.ascontiguousarray(h_q[:, :, sl].transpose(1, 0, 2).reshape(F, NT))
        in_maps.append({
            "x8": _pack(e_c, h_c),
            "Wb": np.ascontiguousarray(np.concatenate([we, wn], axis=1)),
            "bb": np.ascontiguousarray(np.concatenate([be, bn], axis=1)),
        })
    res = run_bass_kernel_spmd(
        nc, in_maps, core_ids=list(range(N_CORES)), trace=trace, **kwargs
    )
    full = np.empty((B, 2 * HALF, N_NODES), dtype=np.float32)
    for c in range(N_CORES):
        oc = _unpack(np.asarray(res.results[c]["out"]))
        deq = oc.astype(np.float32) * so[:, None]
        full[:, :, c * NS : (c + 1) * NS] = (
            deq.reshape(2 * HALF, B, NS).transpose(1, 0, 2)
        )
    return full, res


def kernel(h_v=None, h_w=None, e_vw=None, W_e=None, b_e=None, W_n=None, b_n=None):
    full, _ = run(h_w, e_vw, W_e, b_e, W_n, b_n, trace=False)
    return full
